# revision 26
# baseline (speedup 1.0000x reference)
"""Trainium2 Bass kernel for the sparse-MoE block (top-2 of 8 experts).

Strategy: the router (a tiny [T,H]x[H,E] matmul + top-2) and the token
dispatch run on the host; the expert FFNs -- 99.97% of the FLOPs -- run on
8 NeuronCores. Sharding is F-parallel: each core holds a 512-wide slice of
the FFN intermediate dimension for ALL 8 experts, processes every expert's
gathered token group against its slice, and returns a partial down-proj
output. The host sums the 8 partials and scatter-adds into token order
with the routing weights. This is load-balanced regardless of routing.

Matmuls run in bf16 (fp32 PSUM accumulation): same PE rate as fp32r but
half the HBM traffic, which moves the kernel from the memory/compute ridge
to cleanly compute-bound.
"""

import numpy as np
import ml_dtypes

import concourse.bass as bass
import concourse.tile as tile
from concourse import mybir
from concourse.bass_utils import run_bass_kernel_spmd


def _ensure_ntff_hook():
    """On agent images antenv is a stub without axon_hooks; if a caller sets
    BASS_TRACE=1, run_bass_kernel_spmd would die on that import. Register the
    same ctypes hook trn_boot would have, so tracing works (or degrades to a
    no-trace run) instead of crashing. No-op when the real module exists."""
    try:
        import antenv.axon_hooks  # noqa: F401
        return
    except ImportError:
        pass
    try:
        import sys, types
        import antenv
        from trn_agent_boot.trn_boot import _ntff_profile_via_ctypes

        try:
            hook = _ntff_profile_via_ctypes("/opt/axon/libaxon_pjrt.so")
        except Exception:
            hook = None
        mod = types.ModuleType("antenv.axon_hooks")
        mod.get_axon_ntff_profile_hook = lambda: hook
        mod.set_axon_ntff_profile_hook = lambda h: None
        sys.modules["antenv.axon_hooks"] = mod
        antenv.axon_hooks = mod
    except Exception:
        pass


_ensure_ntff_hook()

B, S, H, F, E = 2, 2048, 1024, 4096, 8
TOP_K = 2
NCORES = 8
FS = F // NCORES  # 512
BF16 = mybir.dt.bfloat16
F32 = mybir.dt.float32
SILU = mybir.ActivationFunctionType.Silu
MULT = mybir.AluOpType.mult
NP_BF16 = ml_dtypes.bfloat16

# Exposed for test harnesses: the BassKernelResults of the last device run
# (carries exec_time_ns + trace path when BASS_TRACE=1).
LAST_RESULTS = None


def _split_multi_waits(nc, max_waits=1):
    """This toolchain's walrus codegen supports one sync-wait per
    instruction; Tile attaches as many as needed. Hoist extras onto
    standalone NoOps just before the instruction on the same engine
    (engine streams execute in order, so semantics are preserved)."""
    total = 0
    for f in nc.m.functions:
        for bb in f.blocks:
            new_insts = []
            changed = False
            for inst in bb.instructions:
                si = inst.sync_info
                waits = list(si.on_wait) if si and si.on_wait else []
                if len(waits) > max_waits:
                    for w in waits[:-max_waits]:
                        nop = mybir.InstNoOp(
                            name=nc.get_next_instruction_name(), ins=[], outs=[]
                        )
                        nop.engine = inst.engine
                        nop.sync_info = mybir.SyncInfo(on_wait=[w], on_update=[])
                        new_insts.append(nop)
                        total += 1
                    inst.sync_info = mybir.SyncInfo(
                        on_wait=waits[-max_waits:],
                        on_update=list(si.on_update) if si.on_update else [],
                    )
                    changed = True
                new_insts.append(inst)
            if changed:
                bb.instructions = new_insts
    return total


def _expert_chunk_widths(cnt):
    # Split a token count into chunk widths <=512 (PSUM bank limit).
    if cnt == 0:
        return []
    n512, tail = divmod(cnt, 512)
    out = [512] * n512
    if tail:
        out.append(tail)
    return out


def _make_chunks(pads):
    chunks = []
    base = 0
    for e, pad in enumerate(pads):
        off = 0
        for w in _expert_chunk_widths(pad):
            chunks.append((e, base + off, w))
            off += w
        base += pad
    # Shrink the drain tail: the final chunk's stage-B runs after all other
    # compute, so make it small by splitting off a 128-token last chunk.
    if chunks and chunks[-1][2] > 128:
        e, c0, w = chunks[-1]
        cut = 128
        chunks[-1] = (e, c0, w - cut)
        chunks.append((e, c0 + w - cut, cut))
    return chunks, base


def _build_program(pads, bufs=None, w_eng='sync', xt_eng='scalar', y_eng='sync', ydt=BF16):
    bufs = {**{'w': 3, 'x': 3, 'a': 3, 'g': 3, 'y': 3, 'pg': 1, 'pu': 5, 'py': 2}, **(bufs or {})}
    chunks, CT = _make_chunks(pads)
    nc = bass.Bass("TRN2", target_bir_lowering=False, debug=False, num_devices=NCORES)
    # Weights are host-packed partition-major so every DMA moves one large
    # contiguous run per partition (descriptor issue rate, not bytes, bounds
    # DMA throughput here). xt stays [H, CT]: a partition-major xt layout
    # would need stride-8 matmul reads, which real PE hardware runs ~3x
    # slower (measured), so xt keeps contiguous token columns instead and
    # gets its own DMA queue.
    xt = nc.declare_dram_parameter("xt", [H, CT], BF16, isOutput=False)
    wg = nc.declare_dram_parameter("wg", [E, 128, (H // 128) * FS], BF16, isOutput=False)
    wu = nc.declare_dram_parameter("wu", [E, 128, (H // 128) * FS], BF16, isOutput=False)
    wd = nc.declare_dram_parameter("wd", [E, 128, (FS // 128) * H], BF16, isOutput=False)
    yp = nc.declare_dram_parameter("yp", [CT, H], ydt, isOutput=True)

    xt3 = xt[:].rearrange("(ko p) c -> p ko c", p=128)  # [128, 8, CT]

    with tile.TileContext(nc) as tc:
        with (
            tc.tile_pool(name="wpool", bufs=bufs["w"]) as wpool,
            tc.tile_pool(name="xpool", bufs=bufs["x"]) as xpool,
            tc.tile_pool(name="apool", bufs=bufs["a"]) as apool,
            tc.tile_pool(name="gpool", bufs=bufs["g"]) as gpool,
            tc.tile_pool(name="ypool", bufs=bufs["y"]) as ypool,
            tc.tile_pool(name="pga", bufs=bufs["pg"], space="PSUM") as pg_pool,
            tc.tile_pool(name="pua", bufs=bufs["pu"], space="PSUM") as pu_pool,
            tc.tile_pool(name="pyb", bufs=bufs["py"], space="PSUM") as py_pool,
        ):
            def load_weights(e, split=False):
                wgt = wpool.tile([128, H // 128, FS], BF16, tag="wg")
                wut = wpool.tile([128, H // 128, FS], BF16, tag="wu")
                wdt = wpool.tile([128, FS // 128, H], BF16, tag="wd")
                weng = getattr(nc, w_eng)
                if split:
                    # Startup only: per-k loads on BOTH HW queues so the first
                    # gate matmul begins after 128KB, and gate/up stream in
                    # parallel instead of serially on one queue.
                    xeng = getattr(nc, xt_eng)
                    for k in range(H // 128):
                        weng.dma_start(wgt[:, k], wg[e, :, k * FS : (k + 1) * FS])
                        xeng.dma_start(wut[:, k], wu[e, :, k * FS : (k + 1) * FS])
                else:
                    weng.dma_start(wgt[:], wg[e].rearrange("p (ko f) -> p ko f", ko=H // 128))
                    weng.dma_start(wut[:], wu[e].rearrange("p (ko f) -> p ko f", ko=H // 128))
                weng.dma_start(wdt[:], wd[e].rearrange("p (ko h) -> p ko h", ko=FS // 128))
                return wgt, wut, wdt

            yodd = [0]

            def stage_b(act, w, c0, wdt):
                for cs in range(-(-w // 128)):
                    m = min(128, w - cs * 128)
                    yt = ypool.tile([128, H], ydt, tag="y")
                    for ht in range(2):
                        py = py_pool.tile([128, 512], F32, tag="py")
                        for kf in range(FS // 128):
                            nc.tensor.matmul(
                                py[:m],
                                act[:, kf, cs * 128 : cs * 128 + m],
                                wdt[:, kf, ht * 512 : (ht + 1) * 512],
                                start=(kf == 0),
                                stop=(kf == FS // 128 - 1),
                            )
                        nc.vector.tensor_copy(yt[:m, ht * 512 : (ht + 1) * 512], py[:m])
                    if y_eng == 'alt':
                        eng = ('sync', 'scalar')[yodd[0] & 1]
                        yodd[0] += 1
                    else:
                        eng = y_eng
                    getattr(nc, eng).dma_start(
                        yp[c0 + cs * 128 : c0 + cs * 128 + m, :], yt[:m]
                    )

            cur_e = -1
            wgt = wut = wdt = None
            prev = None
            first = True
            for e, c0, w in chunks:
                xtile = xpool.tile([128, H // 128, 512], BF16, tag="xt")
                getattr(nc, xt_eng).dma_start(xtile[:, :, :w], xt3[:, :, c0 : c0 + w])
                if e != cur_e:
                    wgt, wut, wdt = load_weights(e, split=first)
                    cur_e = e
                    first = False
                act = apool.tile([128, FS // 128, 512], BF16, tag="act")
                for ft in range(FS // 128):
                    pg = pg_pool.tile([128, 512], F32, tag="pg")
                    pu = pu_pool.tile([128, 512], F32, tag="pu")
                    for k in range(H // 128):
                        nc.tensor.matmul(
                            pg[:, :w],
                            wgt[:, k, ft * 128 : (ft + 1) * 128],
                            xtile[:, k, :w],
                            start=(k == 0),
                            stop=(k == H // 128 - 1),
                        )
                    for k in range(H // 128):
                        nc.tensor.matmul(
                            pu[:, :w],
                            wut[:, k, ft * 128 : (ft + 1) * 128],
                            xtile[:, k, :w],
                            start=(k == 0),
                            stop=(k == H // 128 - 1),
                        )
                    gs = gpool.tile([128, 512], BF16, tag="g")
                    nc.scalar.activation(gs[:, :w], pg[:, :w], SILU)
                    nc.vector.tensor_tensor(act[:, ft, :w], gs[:, :w], pu[:, :w], MULT)
                if prev is not None:
                    stage_b(*prev)
                prev = (act, w, c0, wdt)
            stage_b(*prev)

    _split_multi_waits(nc)
    return nc, CT


_program_cache = {}


def _build_kwargs():
    """Dev A/B knobs; all defaults when env unset (the graded path)."""
    import os

    kw = {}
    if os.environ.get("KMOE_Y_F32"):
        kw["ydt"] = F32
    if os.environ.get("KMOE_WBUFS"):
        kw["bufs"] = {"w": int(os.environ["KMOE_WBUFS"])}
    if os.environ.get("KMOE_YENG"):
        kw["y_eng"] = os.environ["KMOE_YENG"]
    return kw


def _get_program(pads):
    kw = _build_kwargs()
    key = (tuple(pads), tuple(sorted((k, str(v)) for k, v in kw.items())))
    if key not in _program_cache:
        _program_cache[key] = _build_program(pads, **kw)
    return _program_cache[key]


def _route(x, w_gate):
    """Host router: softmax(fp32) then top-2, matching jax.lax.top_k
    tie-breaking (lowest index first)."""
    logits = x @ w_gate  # [T, E] fp32
    m = logits.max(axis=-1, keepdims=True)
    p = np.exp(logits - m, dtype=np.float32)
    p /= p.sum(axis=-1, keepdims=True)
    order = np.argsort(-p, axis=-1, kind="stable")
    sel = order[:, :TOP_K]
    rw = np.take_along_axis(p, sel, axis=-1).astype(np.float32)
    return sel, rw


def kernel(hidden_states, w_gate, w_gate_proj, w_up_proj, w_down_proj):
    global LAST_RESULTS
    x = np.asarray(hidden_states, dtype=np.float32).reshape(-1, H)
    w_gate = np.asarray(w_gate, dtype=np.float32)
    WG = np.asarray(w_gate_proj, dtype=np.float32)
    WU = np.asarray(w_up_proj, dtype=np.float32)
    WD = np.asarray(w_down_proj, dtype=np.float32)
    T = x.shape[0]

    sel, rw = _route(x, w_gate)

    idx, wtok, cnts = [], [], []
    for e in range(E):
        mask0 = sel[:, 0] == e
        mask1 = sel[:, 1] == e
        ie = np.nonzero(mask0 | mask1)[0]
        idx.append(ie)
        wtok.append(np.where(mask0[ie], rw[ie, 0], rw[ie, 1]).astype(np.float32))
        cnts.append(len(ie))

    # Round each expert's token count up to even (the pad column is zeros)
    # so chunk widths stay even for DMA friendliness.
    ecnts = [c + (c & 1) for c in cnts]
    nc, CT = _get_program(ecnts)

    base = np.concatenate([[0], np.cumsum(ecnts)])
    xt = np.zeros((H, CT), dtype=NP_BF16)
    for e in range(E):
        if cnts[e]:
            xt[:, base[e] : base[e] + cnts[e]] = x[idx[e]].T.astype(NP_BF16)

    WGb = WG.astype(NP_BF16)
    WUb = WU.astype(NP_BF16)
    WDb = WD.astype(NP_BF16)

    def pack_hf(w):  # [E, H, FS] -> [E, 128, (H/128)*FS], partition-major
        return np.ascontiguousarray(
            w.reshape(E, H // 128, 128, FS).transpose(0, 2, 1, 3).reshape(E, 128, -1)
        )

    def pack_fh(w):  # [E, FS, H] -> [E, 128, (FS/128)*H], partition-major
        return np.ascontiguousarray(
            w.reshape(E, FS // 128, 128, H).transpose(0, 2, 1, 3).reshape(E, 128, -1)
        )

    in_maps = []
    for c in range(NCORES):
        in_maps.append(
            {
                "xt": xt,
                "wg": pack_hf(WGb[:, :, c * FS : (c + 1) * FS]),
                "wu": pack_hf(WUb[:, :, c * FS : (c + 1) * FS]),
                "wd": pack_fh(WDb[:, c * FS : (c + 1) * FS, :]),
            }
        )
    res = run_bass_kernel_spmd(nc, in_maps, list(range(NCORES)))
    LAST_RESULTS = res

    ysum = res.results[0]["yp"].astype(np.float32)
    for i in range(1, NCORES):
        ysum = ysum + res.results[i]["yp"].astype(np.float32)

    out = np.zeros((T, H), dtype=np.float32)
    for e in range(E):
        if cnts[e]:
            out[idx[e]] += ysum[base[e] : base[e] + cnts[e]] * wtok[e][:, None]
    return out.reshape(B, S, H).astype(np.float32)


# revision 30
# speedup vs baseline: 1.0195x; 1.0195x over previous
"""Trainium2 Bass kernel for the sparse-MoE block (top-2 of 8 experts).

Strategy: the router (a tiny [T,H]x[H,E] matmul + top-2) and the token
dispatch run on the host; the expert FFNs -- 99.97% of the FLOPs -- run on
8 NeuronCores. Sharding is F-parallel: each core holds a 512-wide slice of
the FFN intermediate dimension for ALL 8 experts, processes every expert's
gathered token group against its slice, and returns a partial down-proj
output. The host sums the 8 partials and scatter-adds into token order
with the routing weights. This is load-balanced regardless of routing.

Matmuls run in bf16 (fp32 PSUM accumulation): same PE rate as fp32r but
half the HBM traffic, which moves the kernel from the memory/compute ridge
to cleanly compute-bound.
"""

import numpy as np
import ml_dtypes

import concourse.bass as bass
import concourse.tile as tile
from concourse import mybir
from concourse.bass_utils import run_bass_kernel_spmd


def _ensure_ntff_hook():
    """On agent images antenv is a stub without axon_hooks; if a caller sets
    BASS_TRACE=1, run_bass_kernel_spmd would die on that import. Register the
    same ctypes hook trn_boot would have, so tracing works (or degrades to a
    no-trace run) instead of crashing. No-op when the real module exists."""
    try:
        import antenv.axon_hooks  # noqa: F401
        return
    except ImportError:
        pass
    try:
        import sys, types
        import antenv
        from trn_agent_boot.trn_boot import _ntff_profile_via_ctypes

        try:
            hook = _ntff_profile_via_ctypes("/opt/axon/libaxon_pjrt.so")
        except Exception:
            hook = None
        mod = types.ModuleType("antenv.axon_hooks")
        mod.get_axon_ntff_profile_hook = lambda: hook
        mod.set_axon_ntff_profile_hook = lambda h: None
        sys.modules["antenv.axon_hooks"] = mod
        antenv.axon_hooks = mod
    except Exception:
        pass


_ensure_ntff_hook()

B, S, H, F, E = 2, 2048, 1024, 4096, 8
TOP_K = 2
NCORES = 8
FS = F // NCORES  # 512
BF16 = mybir.dt.bfloat16
F32 = mybir.dt.float32
SILU = mybir.ActivationFunctionType.Silu
MULT = mybir.AluOpType.mult
NP_BF16 = ml_dtypes.bfloat16

# Exposed for test harnesses: the BassKernelResults of the last device run
# (carries exec_time_ns + trace path when BASS_TRACE=1).
LAST_RESULTS = None


def _split_multi_waits(nc, max_waits=1):
    """This toolchain's walrus codegen supports one sync-wait per
    instruction; Tile attaches as many as needed. Hoist extras onto
    standalone NoOps just before the instruction on the same engine
    (engine streams execute in order, so semantics are preserved)."""
    total = 0
    for f in nc.m.functions:
        for bb in f.blocks:
            new_insts = []
            changed = False
            for inst in bb.instructions:
                si = inst.sync_info
                waits = list(si.on_wait) if si and si.on_wait else []
                if len(waits) > max_waits:
                    for w in waits[:-max_waits]:
                        nop = mybir.InstNoOp(
                            name=nc.get_next_instruction_name(), ins=[], outs=[]
                        )
                        nop.engine = inst.engine
                        nop.sync_info = mybir.SyncInfo(on_wait=[w], on_update=[])
                        new_insts.append(nop)
                        total += 1
                    inst.sync_info = mybir.SyncInfo(
                        on_wait=waits[-max_waits:],
                        on_update=list(si.on_update) if si.on_update else [],
                    )
                    changed = True
                new_insts.append(inst)
            if changed:
                bb.instructions = new_insts
    return total


def _expert_chunk_widths(cnt):
    # Split a token count into chunk widths <=512 (PSUM bank limit).
    if cnt == 0:
        return []
    n512, tail = divmod(cnt, 512)
    out = [512] * n512
    if tail:
        out.append(tail)
    return out


def _make_chunks(pads):
    chunks = []
    base = 0
    for e, pad in enumerate(pads):
        off = 0
        for w in _expert_chunk_widths(pad):
            chunks.append((e, base + off, w))
            off += w
        base += pad
    # Shrink the drain tail: the final chunk's stage-B runs after all other
    # compute, so make it small by splitting off a 128-token last chunk.
    if chunks and chunks[-1][2] > 128:
        e, c0, w = chunks[-1]
        cut = 128
        chunks[-1] = (e, c0, w - cut)
        chunks.append((e, c0 + w - cut, cut))
    return chunks, base


def _build_program(pads, bufs=None, w_eng='sync', xt_eng='scalar', y_eng='sync', ydt=BF16,
                   warmup=50):
    bufs = {**{'w': 3, 'x': 3, 'a': 3, 'g': 3, 'y': 3, 'pg': 1, 'pu': 5, 'py': 2}, **(bufs or {})}
    chunks, CT = _make_chunks(pads)
    nc = bass.Bass("TRN2", target_bir_lowering=False, debug=False, num_devices=NCORES)
    # Weights are host-packed partition-major so every DMA moves one large
    # contiguous run per partition (descriptor issue rate, not bytes, bounds
    # DMA throughput here). xt stays [H, CT]: a partition-major xt layout
    # would need stride-8 matmul reads, which real PE hardware runs ~3x
    # slower (measured), so xt keeps contiguous token columns instead and
    # gets its own DMA queue.
    xt = nc.declare_dram_parameter("xt", [H, CT], BF16, isOutput=False)
    wg = nc.declare_dram_parameter("wg", [E, 128, (H // 128) * FS], BF16, isOutput=False)
    wu = nc.declare_dram_parameter("wu", [E, 128, (H // 128) * FS], BF16, isOutput=False)
    wd = nc.declare_dram_parameter("wd", [E, 128, (FS // 128) * H], BF16, isOutput=False)
    yp = nc.declare_dram_parameter("yp", [CT, H], ydt, isOutput=True)

    xt3 = xt[:].rearrange("(ko p) c -> p ko c", p=128)  # [128, 8, CT]

    with tile.TileContext(nc) as tc:
        with (
            tc.tile_pool(name="wpool", bufs=bufs["w"]) as wpool,
            tc.tile_pool(name="xpool", bufs=bufs["x"]) as xpool,
            tc.tile_pool(name="apool", bufs=bufs["a"]) as apool,
            tc.tile_pool(name="gpool", bufs=bufs["g"]) as gpool,
            tc.tile_pool(name="ypool", bufs=bufs["y"]) as ypool,
            tc.tile_pool(name="pga", bufs=bufs["pg"], space="PSUM") as pg_pool,
            tc.tile_pool(name="pua", bufs=bufs["pu"], space="PSUM") as pu_pool,
            tc.tile_pool(name="pyb", bufs=bufs["py"], space="PSUM") as py_pool,
        ):
            def load_weights(e, split=False):
                wgt = wpool.tile([128, H // 128, FS], BF16, tag="wg")
                wut = wpool.tile([128, H // 128, FS], BF16, tag="wu")
                wdt = wpool.tile([128, FS // 128, H], BF16, tag="wd")
                weng = getattr(nc, w_eng)
                if split:
                    # Startup only: per-k loads so the first gate matmul can
                    # begin after 128KB instead of the full 1MB tile.
                    for k in range(H // 128):
                        weng.dma_start(wgt[:, k], wg[e, :, k * FS : (k + 1) * FS])
                    for k in range(H // 128):
                        weng.dma_start(wut[:, k], wu[e, :, k * FS : (k + 1) * FS])
                else:
                    weng.dma_start(wgt[:], wg[e].rearrange("p (ko f) -> p ko f", ko=H // 128))
                    weng.dma_start(wut[:], wu[e].rearrange("p (ko f) -> p ko f", ko=H // 128))
                weng.dma_start(wdt[:], wd[e].rearrange("p (ko h) -> p ko h", ko=FS // 128))
                return wgt, wut, wdt

            yodd = [0]

            def stage_b(act, w, c0, wdt):
                for cs in range(-(-w // 128)):
                    m = min(128, w - cs * 128)
                    yt = ypool.tile([128, H], ydt, tag="y")
                    for ht in range(2):
                        py = py_pool.tile([128, 512], F32, tag="py")
                        for kf in range(FS // 128):
                            nc.tensor.matmul(
                                py[:m],
                                act[:, kf, cs * 128 : cs * 128 + m],
                                wdt[:, kf, ht * 512 : (ht + 1) * 512],
                                start=(kf == 0),
                                stop=(kf == FS // 128 - 1),
                            )
                        nc.vector.tensor_copy(yt[:m, ht * 512 : (ht + 1) * 512], py[:m])
                    if y_eng == 'alt':
                        eng = ('sync', 'scalar')[yodd[0] & 1]
                        yodd[0] += 1
                    else:
                        eng = y_eng
                    getattr(nc, eng).dma_start(
                        yp[c0 + cs * 128 : c0 + cs * 128 + m, :], yt[:m]
                    )

            # PE warmup: dummy matmuls on a zeroed tile while the first
            # weight/x DMAs are in flight. Keeps the Tensor engine busy from
            # t~1us so its DVFS p-state is at full clock when real work
            # arrives (idle PE restarts at half clock for ~3us).
            if warmup:
                wz = gpool.tile([128, 512], BF16, tag="wz")
                nc.vector.memset(wz[:], 0.0)
                pz = py_pool.tile([128, 512], F32, tag="py")
                for i in range(warmup):
                    nc.tensor.matmul(
                        pz[:], wz[:, :128], wz[:], start=(i == 0), stop=(i == warmup - 1)
                    )

            cur_e = -1
            wgt = wut = wdt = None
            prev = None
            first = True
            for e, c0, w in chunks:
                xtile = xpool.tile([128, H // 128, 512], BF16, tag="xt")
                getattr(nc, xt_eng).dma_start(xtile[:, :, :w], xt3[:, :, c0 : c0 + w])
                if e != cur_e:
                    wgt, wut, wdt = load_weights(e, split=first)
                    cur_e = e
                    first = False
                act = apool.tile([128, FS // 128, 512], BF16, tag="act")
                for ft in range(FS // 128):
                    pg = pg_pool.tile([128, 512], F32, tag="pg")
                    pu = pu_pool.tile([128, 512], F32, tag="pu")
                    for k in range(H // 128):
                        nc.tensor.matmul(
                            pg[:, :w],
                            wgt[:, k, ft * 128 : (ft + 1) * 128],
                            xtile[:, k, :w],
                            start=(k == 0),
                            stop=(k == H // 128 - 1),
                        )
                    for k in range(H // 128):
                        nc.tensor.matmul(
                            pu[:, :w],
                            wut[:, k, ft * 128 : (ft + 1) * 128],
                            xtile[:, k, :w],
                            start=(k == 0),
                            stop=(k == H // 128 - 1),
                        )
                    gs = gpool.tile([128, 512], BF16, tag="g")
                    nc.scalar.activation(gs[:, :w], pg[:, :w], SILU)
                    nc.vector.tensor_tensor(act[:, ft, :w], gs[:, :w], pu[:, :w], MULT)
                if prev is not None:
                    stage_b(*prev)
                prev = (act, w, c0, wdt)
            stage_b(*prev)

    _split_multi_waits(nc)
    return nc, CT


_program_cache = {}


def _build_kwargs():
    """Dev A/B knobs; all defaults when env unset (the graded path)."""
    import os

    kw = {}
    if os.environ.get("KMOE_Y_F32"):
        kw["ydt"] = F32
    if os.environ.get("KMOE_WBUFS"):
        kw["bufs"] = {"w": int(os.environ["KMOE_WBUFS"])}
    if os.environ.get("KMOE_YENG"):
        kw["y_eng"] = os.environ["KMOE_YENG"]
    if os.environ.get("KMOE_WARMUP"):
        kw["warmup"] = int(os.environ["KMOE_WARMUP"])
    if os.environ.get("KMOE_PSUM"):
        pg, pu, py = (int(v) for v in os.environ["KMOE_PSUM"].split(","))
        kw.setdefault("bufs", {}).update({"pg": pg, "pu": pu, "py": py})
    return kw


def _get_program(pads):
    kw = _build_kwargs()
    key = (tuple(pads), tuple(sorted((k, str(v)) for k, v in kw.items())))
    if key not in _program_cache:
        _program_cache[key] = _build_program(pads, **kw)
    return _program_cache[key]


def _route(x, w_gate):
    """Host router: softmax(fp32) then top-2, matching jax.lax.top_k
    tie-breaking (lowest index first)."""
    logits = x @ w_gate  # [T, E] fp32
    m = logits.max(axis=-1, keepdims=True)
    p = np.exp(logits - m, dtype=np.float32)
    p /= p.sum(axis=-1, keepdims=True)
    order = np.argsort(-p, axis=-1, kind="stable")
    sel = order[:, :TOP_K]
    rw = np.take_along_axis(p, sel, axis=-1).astype(np.float32)
    return sel, rw


def kernel(hidden_states, w_gate, w_gate_proj, w_up_proj, w_down_proj):
    global LAST_RESULTS
    x = np.asarray(hidden_states, dtype=np.float32).reshape(-1, H)
    w_gate = np.asarray(w_gate, dtype=np.float32)
    WG = np.asarray(w_gate_proj, dtype=np.float32)
    WU = np.asarray(w_up_proj, dtype=np.float32)
    WD = np.asarray(w_down_proj, dtype=np.float32)
    T = x.shape[0]

    sel, rw = _route(x, w_gate)

    idx, wtok, cnts = [], [], []
    for e in range(E):
        mask0 = sel[:, 0] == e
        mask1 = sel[:, 1] == e
        ie = np.nonzero(mask0 | mask1)[0]
        idx.append(ie)
        wtok.append(np.where(mask0[ie], rw[ie, 0], rw[ie, 1]).astype(np.float32))
        cnts.append(len(ie))

    # Round each expert's token count up to even (the pad column is zeros)
    # so chunk widths stay even for DMA friendliness.
    ecnts = [c + (c & 1) for c in cnts]
    nc, CT = _get_program(ecnts)

    base = np.concatenate([[0], np.cumsum(ecnts)])
    xt = np.zeros((H, CT), dtype=NP_BF16)
    for e in range(E):
        if cnts[e]:
            xt[:, base[e] : base[e] + cnts[e]] = x[idx[e]].T.astype(NP_BF16)

    WGb = WG.astype(NP_BF16)
    WUb = WU.astype(NP_BF16)
    WDb = WD.astype(NP_BF16)

    def pack_hf(w):  # [E, H, FS] -> [E, 128, (H/128)*FS], partition-major
        return np.ascontiguousarray(
            w.reshape(E, H // 128, 128, FS).transpose(0, 2, 1, 3).reshape(E, 128, -1)
        )

    def pack_fh(w):  # [E, FS, H] -> [E, 128, (FS/128)*H], partition-major
        return np.ascontiguousarray(
            w.reshape(E, FS // 128, 128, H).transpose(0, 2, 1, 3).reshape(E, 128, -1)
        )

    in_maps = []
    for c in range(NCORES):
        in_maps.append(
            {
                "xt": xt,
                "wg": pack_hf(WGb[:, :, c * FS : (c + 1) * FS]),
                "wu": pack_hf(WUb[:, :, c * FS : (c + 1) * FS]),
                "wd": pack_fh(WDb[:, c * FS : (c + 1) * FS, :]),
            }
        )
    res = run_bass_kernel_spmd(nc, in_maps, list(range(NCORES)))
    LAST_RESULTS = res

    ysum = res.results[0]["yp"].astype(np.float32)
    for i in range(1, NCORES):
        ysum = ysum + res.results[i]["yp"].astype(np.float32)

    out = np.zeros((T, H), dtype=np.float32)
    for e in range(E):
        if cnts[e]:
            out[idx[e]] += ysum[base[e] : base[e] + cnts[e]] * wtok[e][:, None]
    return out.reshape(B, S, H).astype(np.float32)
